# revision 16
# baseline (speedup 1.0000x reference)
"""MoE layer (B=2,S=1024,H=2048,F=5504,E=8,top-2) on 8 NeuronCores.

Strategy: expert-parallel. Host computes the (tiny) router + top-2 dispatch,
gathers each expert's tokens, and feeds core e exactly expert e's weights and
tokens. Each core runs a bf16 SwiGLU MLP (fp32 PSUM accumulation) over its
padded token set, scales rows by the combine probabilities, and the host
scatter-adds the per-expert outputs back into the full [T, H] output.
"""

import sys

import numpy as np
import ml_dtypes

if "/opt/trn_rl_repo" not in sys.path:
    sys.path.insert(0, "/opt/trn_rl_repo")

B, S, H, F, E, TOPK = 2, 1024, 2048, 5504, 8, 2
T = B * S
P = 128
FT = F // P  # 43 f-tiles of 128
HC = H // P  # 16 h-chunks of 128
NCORES = 8
BF16 = ml_dtypes.bfloat16

_nc_cache: dict = {}

# test-harness knobs (harmless defaults for grading)
TRACE = False
LAST_RESULT = None


def _split_waits(nc):
    """Walrus on this toolchain encodes at most ONE sync wait per DMA-queue
    instruction (DIRECT2D EVENTS has a single wait slot) and refuses
    multi-wait drains. Tile emits multi-wait DMAs for slot-reuse (WAR vs
    readers + WAW vs previous fill). Fix up the scheduled BIR: for each
    queue DMA with N>1 waits, insert N-1 zero-update 2-byte scratch DMAs
    ("wait gates") on the same queue immediately before it, each carrying
    one wait — queue FIFO issue makes the semantics identical. Multi-wait
    drains are split into single-wait drain clones the same way."""
    import copy
    import concourse.mybir as mybir

    tmpl = None
    for f in nc.m.functions:
        for b in f.blocks:
            for ins in b.instructions:
                if type(ins).__name__ == "InstDMACopy" and ins.outs and "wgdst" in str(
                    ins.outs[0]
                ):
                    tmpl = ins
    assert tmpl is not None, "wait-gate template (wgdst dma) not found"

    k = 0
    for f in nc.m.functions:
        for b in f.blocks:
            newlist = []
            for ins in b.instructions:
                si = ins.sync_info
                tn = type(ins).__name__
                if (
                    tn == "InstDMACopy"
                    and ins.name != tmpl.name
                    and si is not None
                    and si.on_wait
                    and len(si.on_wait) > 1
                ):
                    waits = list(si.on_wait)
                    for w in waits[:-1]:
                        k += 1
                        upd = copy.deepcopy(list(tmpl.sync_info.on_update))
                        for u in upd:
                            u.update_value = 0
                        d = mybir.InstDMACopy(
                            name=f"I-{900000 + k}",
                            engine=tmpl.engine,
                            ins=copy.deepcopy(tmpl.ins),
                            outs=copy.deepcopy(tmpl.outs),
                            queue=getattr(ins, "queue", None) or tmpl.queue,
                            mode=tmpl.mode,
                            oob_is_err=tmpl.oob_is_err,
                            cce_op=tmpl.cce_op,
                            single_packet=tmpl.single_packet,
                            sync_info=mybir.SyncInfo(on_wait=[w], on_update=upd),
                        )
                        newlist.append(d)
                    ins.sync_info = mybir.SyncInfo(
                        on_wait=[waits[-1]], on_update=list(si.on_update or [])
                    )
                elif si is not None and si.on_wait and len(si.on_wait) > 1:
                    waits = list(si.on_wait)
                    for w in waits[:-1]:
                        k += 1
                        d = mybir.InstEventSemaphore(
                            name=f"I-{900000 + k}",
                            engine=ins.engine,
                            sync_info=mybir.SyncInfo(on_wait=[w], on_update=[]),
                        )
                        newlist.append(d)
                    ins.sync_info = mybir.SyncInfo(
                        on_wait=[waits[-1]], on_update=list(si.on_update or [])
                    )
                newlist.append(ins)
            b.instructions[:] = newlist
    return k


def _build_nc(npad: int, dt_eng: str = "sync", gu_eng: str = "sync", gu_bufs: int = 4):
    import concourse.bass as bass
    import concourse.mybir as mybir
    from concourse.tile import TileContext
    from contextlib import ExitStack

    dt = mybir.dt
    MT = npad // P  # token m-tiles (lhsT partition groups for down proj)
    HTILES = H // 512  # 4

    # token N-tiles for the gate/up matmuls (PSUM bank limit: 512 fp32)
    ttiles = []
    off = 0
    while off < npad:
        n = min(512, npad - off)
        ttiles.append((off, n))
        off += n

    # f-chunk groups for the down matmul: accumulate 9 chunks in PSUM, then
    # add into the fp32 SBUF accumulator (43 chunks don't fit PSUM/SBUF).
    GRP = 9
    groups = [list(range(s, min(s + GRP, FT))) for s in range(0, FT, GRP)]

    nc = bass.Bass()
    xt = nc.dram_tensor("xt", [P, HC, npad], dt.bfloat16, kind="ExternalInput")
    gt = nc.dram_tensor("gt", [FT, P, HC * P], dt.bfloat16, kind="ExternalInput")
    ut = nc.dram_tensor("ut", [FT, P, HC * P], dt.bfloat16, kind="ExternalInput")
    dw = nc.dram_tensor("dw", [FT, P, H], dt.bfloat16, kind="ExternalInput")
    pr = nc.dram_tensor("pr", [P, MT], dt.float32, kind="ExternalInput")
    out = nc.dram_tensor("out", [MT, P, H], dt.float32, kind="ExternalOutput")
    wgsrc = nc.dram_tensor("wgsrc", [1, 1], dt.bfloat16, kind="ExternalInput")
    wgdst = nc.dram_tensor("wgdst", [1, 1], dt.bfloat16)

    with TileContext(nc) as tc, ExitStack() as ctx:
        cpool = ctx.enter_context(tc.tile_pool(name="const", bufs=1))
        gpool = ctx.enter_context(tc.tile_pool(name="gw", bufs=gu_bufs))
        upool = ctx.enter_context(tc.tile_pool(name="uw", bufs=gu_bufs))
        dpool = ctx.enter_context(tc.tile_pool(name="dwp", bufs=GRP))
        spool = ctx.enter_context(tc.tile_pool(name="stmp", bufs=2))
        p512 = ctx.enter_context(tc.tile_pool(name="p512", bufs=2, space="PSUM"))
        p128 = ctx.enter_context(tc.tile_pool(name="p128", bufs=1, space="PSUM"))
        pdn = ctx.enter_context(tc.tile_pool(name="pdn", bufs=2, space="PSUM"))

        x_sb = cpool.tile([P, HC, npad], dt.bfloat16, tag="x")
        nc.sync.dma_start(x_sb[:], xt[:])
        pr_sb = cpool.tile([P, MT], dt.float32, tag="pr")
        nc.sync.dma_start(pr_sb[:], pr[:])
        h_sb = cpool.tile([P, FT, npad], dt.bfloat16, tag="h")
        y_sb = cpool.tile([P, MT, H], dt.float32, tag="y")

        def emit_down_group(gi: int):
            grp = groups[gi]
            dts = []
            for fc in grp:
                dtile = dpool.tile([P, H], dt.bfloat16, tag="dw")
                getattr(nc, dt_eng).dma_start(dtile[:], dw[fc])
                dts.append(dtile)
            for m in range(MT):
                for ht in range(HTILES):
                    ps = pdn.tile([P, 512], dt.float32, tag="dn")
                    for j, fc in enumerate(grp):
                        nc.tensor.matmul(
                            ps[:],
                            h_sb[:, fc, m * P : (m + 1) * P],
                            dts[j][:, ht * 512 : (ht + 1) * 512],
                            start=(j == 0),
                            stop=(j == len(grp) - 1),
                        )
                    dst = y_sb[:, m, ht * 512 : (ht + 1) * 512]
                    if gi == 0:
                        nc.vector.tensor_copy(dst, ps[:])
                    else:
                        nc.vector.tensor_add(out=dst, in0=dst, in1=ps[:])

        next_grp = 0
        for ft in range(FT):
            g_sl = gpool.tile([P, HC * P], dt.bfloat16, tag="g")
            getattr(nc, gu_eng).dma_start(g_sl[:], gt[ft])
            u_sl = upool.tile([P, HC * P], dt.bfloat16, tag="u")
            getattr(nc, gu_eng).dma_start(u_sl[:], ut[ft])

            psg, psu = {}, {}
            for t0, n in ttiles:
                pool = p512 if n == 512 else p128
                gp = pool.tile([P, n], dt.float32, tag=f"g{n}")
                for hc in range(HC):
                    nc.tensor.matmul(
                        gp[:],
                        g_sl[:, hc * P : (hc + 1) * P],
                        x_sb[:, hc, t0 : t0 + n],
                        start=(hc == 0),
                        stop=(hc == HC - 1),
                    )
                psg[t0] = gp
            for t0, n in ttiles:
                pool = p512 if n == 512 else p128
                up = pool.tile([P, n], dt.float32, tag=f"u{n}")
                for hc in range(HC):
                    nc.tensor.matmul(
                        up[:],
                        u_sl[:, hc * P : (hc + 1) * P],
                        x_sb[:, hc, t0 : t0 + n],
                        start=(hc == 0),
                        stop=(hc == HC - 1),
                    )
                psu[t0] = up
            for t0, n in ttiles:
                st = spool.tile([P, 512], dt.float32, tag="st")
                nc.scalar.activation(
                    st[:, :n], psg[t0][:], mybir.ActivationFunctionType.Sigmoid
                )
                nc.vector.tensor_mul(out=st[:, :n], in0=st[:, :n], in1=psg[t0][:])
                nc.vector.tensor_mul(
                    out=h_sb[:, ft, t0 : t0 + n], in0=st[:, :n], in1=psu[t0][:]
                )

            # interleave down-proj groups as soon as their h chunks are ready
            if next_grp < len(groups) and ft == groups[next_grp][-1]:
                emit_down_group(next_grp)
                next_grp += 1

        for m in range(MT):
            nc.vector.tensor_scalar_mul(
                y_sb[:, m, :], y_sb[:, m, :], pr_sb[:, m : m + 1]
            )
            nc.sync.dma_start(out[m], y_sb[:, m, :])

        # template for the wait-gate post-pass (see _split_waits)
        nc.sync.dma_start(wgdst[:], wgsrc[:])

    _split_waits(nc)
    return nc


def _route(xf: np.ndarray, router_w: np.ndarray):
    """Top-2 routing, reproducing jax.lax.top_k (ties -> lower index) and
    softmax over the two selected logits."""
    logits = xf.astype(np.float64) @ router_w.astype(np.float64).T  # [T, E]
    order = np.argsort(-logits, axis=-1, kind="stable")[:, :TOPK]  # [T, 2]
    top_v = np.take_along_axis(logits, order, axis=1)
    ex = np.exp(top_v - top_v.max(axis=1, keepdims=True))
    probs = (ex / ex.sum(axis=1, keepdims=True)).astype(np.float32)
    return order, probs


def _prep_expert_weights(gate_w, up_w, down_w, e: int):
    g16 = gate_w[e].astype(BF16)  # [F, H]
    u16 = up_w[e].astype(BF16)  # [F, H]
    d16 = down_w[e].astype(BF16)  # [H, F]
    # [ft, hp, hc*128+fi] = w[ft*128+fi, hc*128+hp]
    gtt = np.ascontiguousarray(
        g16.reshape(FT, P, HC, P).transpose(0, 3, 2, 1)
    ).reshape(FT, P, HC * P)
    utt = np.ascontiguousarray(
        u16.reshape(FT, P, HC, P).transpose(0, 3, 2, 1)
    ).reshape(FT, P, HC * P)
    # [fc, fp, h] = down_w[h, fc*128+fp]
    dtt = np.ascontiguousarray(d16.T).reshape(FT, P, H)
    return gtt, utt, dtt


def kernel(x, router_w, gate_w, up_w, down_w):
    from concourse.bass_utils import run_bass_kernel_spmd

    x = np.asarray(x)
    router_w = np.asarray(router_w)
    gate_w = np.asarray(gate_w)
    up_w = np.asarray(up_w)
    down_w = np.asarray(down_w)

    xf = x.reshape(T, H)
    order, probs = _route(xf, router_w)

    # per-expert token lists + combine weights
    idxs, pes = [], []
    for e in range(E):
        sel = (order[:, 0] == e) | (order[:, 1] == e)
        idx = np.nonzero(sel)[0]
        pe = np.where(order[idx, 0] == e, probs[idx, 0], probs[idx, 1])
        idxs.append(idx)
        pes.append(pe.astype(np.float32))

    maxn = max(len(i) for i in idxs)
    npad = max(P, -(-maxn // P) * P)
    MT = npad // P

    if npad not in _nc_cache:
        _nc_cache[npad] = _build_nc(npad)
    nc = _nc_cache[npad]

    in_maps = []
    for e in range(E):
        idx, pe = idxs[e], pes[e]
        xg = np.zeros((npad, H), dtype=BF16)
        xg[: len(idx)] = xf[idx].astype(BF16)
        # [p, hc, t] = xg[t, hc*128+p]
        xtt = np.ascontiguousarray(xg.reshape(npad, HC, P).transpose(2, 1, 0))
        pp = np.zeros(npad, dtype=np.float32)
        pp[: len(idx)] = pe
        prt = np.ascontiguousarray(pp.reshape(MT, P).T)
        gtt, utt, dtt = _prep_expert_weights(gate_w, up_w, down_w, e)
        in_maps.append(
            {
                "xt": xtt,
                "gt": gtt,
                "ut": utt,
                "dw": dtt,
                "pr": prt,
                "wgsrc": np.zeros((1, 1), dtype=BF16),
            }
        )

    res = run_bass_kernel_spmd(
        nc, in_maps, core_ids=list(range(NCORES)), trace=TRACE
    )
    global LAST_RESULT
    LAST_RESULT = res

    out_flat = np.zeros((T, H), dtype=np.float32)
    for e in range(E):
        y = res.results[e]["out"].reshape(npad, H)
        out_flat[idxs[e]] += y[: len(idxs[e])]
    return out_flat.reshape(B, S, H)


# revision 19
# speedup vs baseline: 1.1815x; 1.1815x over previous
"""MoE layer (B=2,S=1024,H=2048,F=5504,E=8,top-2) on 8 NeuronCores.

Strategy: expert-parallel. Host computes the (tiny) router + top-2 dispatch,
gathers each expert's tokens, and feeds core e exactly expert e's weights and
tokens. Each core runs a bf16 SwiGLU MLP (fp32 PSUM accumulation) over its
padded token set, scales rows by the combine probabilities, and the host
scatter-adds the per-expert outputs back into the full [T, H] output.
"""

import sys

import numpy as np
import ml_dtypes

if "/opt/trn_rl_repo" not in sys.path:
    sys.path.insert(0, "/opt/trn_rl_repo")

B, S, H, F, E, TOPK = 2, 1024, 2048, 5504, 8, 2
T = B * S
P = 128
FT = F // P  # 43 f-tiles of 128
HC = H // P  # 16 h-chunks of 128
NCORES = 8
BF16 = ml_dtypes.bfloat16

_nc_cache: dict = {}

# test-harness knobs (harmless defaults for grading)
TRACE = False
LAST_RESULT = None


def _split_waits(nc):
    """Walrus on this toolchain encodes at most ONE sync wait per DMA-queue
    instruction (DIRECT2D EVENTS has a single wait slot) and refuses
    multi-wait drains. Tile emits multi-wait DMAs for slot-reuse (WAR vs
    readers + WAW vs previous fill). Fix up the scheduled BIR: for each
    queue DMA with N>1 waits, insert N-1 zero-update 2-byte scratch DMAs
    ("wait gates") on the same queue immediately before it, each carrying
    one wait — queue FIFO issue makes the semantics identical. Multi-wait
    drains are split into single-wait drain clones the same way."""
    import copy
    import concourse.mybir as mybir

    tmpl = None
    for f in nc.m.functions:
        for b in f.blocks:
            for ins in b.instructions:
                if type(ins).__name__ == "InstDMACopy" and ins.outs and "wgdst" in str(
                    ins.outs[0]
                ):
                    tmpl = ins
    assert tmpl is not None, "wait-gate template (wgdst dma) not found"

    k = 0
    for f in nc.m.functions:
        for b in f.blocks:
            newlist = []
            for ins in b.instructions:
                si = ins.sync_info
                tn = type(ins).__name__
                if (
                    tn == "InstDMACopy"
                    and ins.name != tmpl.name
                    and si is not None
                    and si.on_wait
                    and len(si.on_wait) > 1
                ):
                    waits = list(si.on_wait)
                    for w in waits[:-1]:
                        k += 1
                        upd = copy.deepcopy(list(tmpl.sync_info.on_update))
                        for u in upd:
                            u.update_value = 0
                        d = mybir.InstDMACopy(
                            name=f"I-{900000 + k}",
                            engine=tmpl.engine,
                            ins=copy.deepcopy(tmpl.ins),
                            outs=copy.deepcopy(tmpl.outs),
                            queue=getattr(ins, "queue", None) or tmpl.queue,
                            mode=tmpl.mode,
                            oob_is_err=tmpl.oob_is_err,
                            cce_op=tmpl.cce_op,
                            single_packet=tmpl.single_packet,
                            sync_info=mybir.SyncInfo(on_wait=[w], on_update=upd),
                        )
                        newlist.append(d)
                    ins.sync_info = mybir.SyncInfo(
                        on_wait=[waits[-1]], on_update=list(si.on_update or [])
                    )
                elif si is not None and si.on_wait and len(si.on_wait) > 1:
                    waits = list(si.on_wait)
                    for w in waits[:-1]:
                        k += 1
                        d = mybir.InstEventSemaphore(
                            name=f"I-{900000 + k}",
                            engine=ins.engine,
                            sync_info=mybir.SyncInfo(on_wait=[w], on_update=[]),
                        )
                        newlist.append(d)
                    ins.sync_info = mybir.SyncInfo(
                        on_wait=[waits[-1]], on_update=list(si.on_update or [])
                    )
                newlist.append(ins)
            b.instructions[:] = newlist
    return k


def _build_nc(npad: int, dt_eng: str = "sync", gu_eng: str = "sync", gu_bufs: int = 3):
    import concourse.bass as bass
    import concourse.mybir as mybir
    from concourse.tile import TileContext
    from contextlib import ExitStack

    dt = mybir.dt
    MT = npad // P  # token m-tiles (lhsT partition groups for down proj)
    HTILES = H // 512  # 4

    # token N-tiles for the gate/up matmuls (PSUM bank limit: 512 fp32)
    ttiles = []
    off = 0
    while off < npad:
        n = min(512, npad - off)
        ttiles.append((off, n))
        off += n

    # f-chunk groups for the down matmul: accumulate 9 chunks in PSUM, then
    # add into the fp32 SBUF accumulator (43 chunks don't fit PSUM/SBUF).
    GRP = 9
    groups = [list(range(s, min(s + GRP, FT))) for s in range(0, FT, GRP)]

    nc = bass.Bass()
    xt = nc.dram_tensor("xt", [P, HC, npad], dt.bfloat16, kind="ExternalInput")
    gt = nc.dram_tensor("gt", [FT, P, HC * P], dt.bfloat16, kind="ExternalInput")
    ut = nc.dram_tensor("ut", [FT, P, HC * P], dt.bfloat16, kind="ExternalInput")
    dw = nc.dram_tensor("dw", [FT, P, H], dt.bfloat16, kind="ExternalInput")
    pr = nc.dram_tensor("pr", [P, MT], dt.float32, kind="ExternalInput")
    out = nc.dram_tensor("out", [MT, P, H], dt.float32, kind="ExternalOutput")
    wgsrc = nc.dram_tensor("wgsrc", [1, 1], dt.bfloat16, kind="ExternalInput")
    wgdst = nc.dram_tensor("wgdst", [1, 1], dt.bfloat16)

    with TileContext(nc) as tc, ExitStack() as ctx:
        cpool = ctx.enter_context(tc.tile_pool(name="const", bufs=1))
        gpool = ctx.enter_context(tc.tile_pool(name="gw", bufs=gu_bufs))
        upool = ctx.enter_context(tc.tile_pool(name="uw", bufs=gu_bufs))
        dpool = ctx.enter_context(tc.tile_pool(name="dwp", bufs=GRP + 3))
        spool = ctx.enter_context(tc.tile_pool(name="stmp", bufs=2))
        p512 = ctx.enter_context(tc.tile_pool(name="p512", bufs=2, space="PSUM"))
        p128 = ctx.enter_context(tc.tile_pool(name="p128", bufs=1, space="PSUM"))
        pdn = ctx.enter_context(tc.tile_pool(name="pdn", bufs=2, space="PSUM"))

        x_sb = cpool.tile([P, HC, npad], dt.bfloat16, tag="x")
        for hc in range(HC):
            nc.sync.dma_start(x_sb[:, hc, :], xt[:, hc, :])
        pr_sb = cpool.tile([P, MT], dt.float32, tag="pr")
        nc.sync.dma_start(pr_sb[:], pr[:])
        h_sb = cpool.tile([P, FT, npad], dt.bfloat16, tag="h")
        y_sb = cpool.tile([P, MT, H], dt.float32, tag="y")

        def emit_down_group(gi: int):
            grp = groups[gi]
            dts = []
            for fc in grp:
                dtile = dpool.tile([P, H], dt.bfloat16, tag="dw")
                getattr(nc, dt_eng).dma_start(dtile[:], dw[fc])
                dts.append(dtile)
            for m in range(MT):
                for ht in range(HTILES):
                    ps = pdn.tile([P, 512], dt.float32, tag="dn")
                    for j, fc in enumerate(grp):
                        nc.tensor.matmul(
                            ps[:],
                            h_sb[:, fc, m * P : (m + 1) * P],
                            dts[j][:, ht * 512 : (ht + 1) * 512],
                            start=(j == 0),
                            stop=(j == len(grp) - 1),
                        )
                    dst = y_sb[:, m, ht * 512 : (ht + 1) * 512]
                    if gi == 0:
                        nc.vector.tensor_copy(dst, ps[:])
                    else:
                        nc.vector.tensor_add(out=dst, in0=dst, in1=ps[:])

        next_grp = 0
        for ft in range(FT):
            g_sl = gpool.tile([P, HC * P], dt.bfloat16, tag="g")
            getattr(nc, gu_eng).dma_start(g_sl[:], gt[ft])
            u_sl = upool.tile([P, HC * P], dt.bfloat16, tag="u")
            getattr(nc, gu_eng).dma_start(u_sl[:], ut[ft])

            psg, psu = {}, {}
            for t0, n in ttiles:
                pool = p512 if n == 512 else p128
                gp = pool.tile([P, n], dt.float32, tag=f"g{n}")
                for hc in range(HC):
                    nc.tensor.matmul(
                        gp[:],
                        g_sl[:, hc * P : (hc + 1) * P],
                        x_sb[:, hc, t0 : t0 + n],
                        start=(hc == 0),
                        stop=(hc == HC - 1),
                    )
                psg[t0] = gp
            for t0, n in ttiles:
                pool = p512 if n == 512 else p128
                up = pool.tile([P, n], dt.float32, tag=f"u{n}")
                for hc in range(HC):
                    nc.tensor.matmul(
                        up[:],
                        u_sl[:, hc * P : (hc + 1) * P],
                        x_sb[:, hc, t0 : t0 + n],
                        start=(hc == 0),
                        stop=(hc == HC - 1),
                    )
                psu[t0] = up
            for t0, n in ttiles:
                st = spool.tile([P, 512], dt.float32, tag="st")
                nc.scalar.activation(
                    st[:, :n], psg[t0][:], mybir.ActivationFunctionType.Sigmoid
                )
                nc.vector.tensor_mul(out=st[:, :n], in0=st[:, :n], in1=psg[t0][:])
                nc.vector.tensor_mul(
                    out=h_sb[:, ft, t0 : t0 + n], in0=st[:, :n], in1=psu[t0][:]
                )

            # interleave down-proj groups as soon as their h chunks are ready
            if next_grp < len(groups) and ft == groups[next_grp][-1]:
                emit_down_group(next_grp)
                next_grp += 1

        for m in range(MT):
            nc.vector.tensor_scalar_mul(
                y_sb[:, m, :], y_sb[:, m, :], pr_sb[:, m : m + 1]
            )
            nc.sync.dma_start(out[m], y_sb[:, m, :])

        # template for the wait-gate post-pass (see _split_waits)
        nc.sync.dma_start(wgdst[:], wgsrc[:])

    _split_waits(nc)
    return nc


def _route(xf: np.ndarray, router_w: np.ndarray):
    """Top-2 routing, reproducing jax.lax.top_k (ties -> lower index) and
    softmax over the two selected logits."""
    logits = xf.astype(np.float64) @ router_w.astype(np.float64).T  # [T, E]
    order = np.argsort(-logits, axis=-1, kind="stable")[:, :TOPK]  # [T, 2]
    top_v = np.take_along_axis(logits, order, axis=1)
    ex = np.exp(top_v - top_v.max(axis=1, keepdims=True))
    probs = (ex / ex.sum(axis=1, keepdims=True)).astype(np.float32)
    return order, probs


def _prep_expert_weights(gate_w, up_w, down_w, e: int):
    g16 = gate_w[e].astype(BF16)  # [F, H]
    u16 = up_w[e].astype(BF16)  # [F, H]
    d16 = down_w[e].astype(BF16)  # [H, F]
    # [ft, hp, hc*128+fi] = w[ft*128+fi, hc*128+hp]
    gtt = np.ascontiguousarray(
        g16.reshape(FT, P, HC, P).transpose(0, 3, 2, 1)
    ).reshape(FT, P, HC * P)
    utt = np.ascontiguousarray(
        u16.reshape(FT, P, HC, P).transpose(0, 3, 2, 1)
    ).reshape(FT, P, HC * P)
    # [fc, fp, h] = down_w[h, fc*128+fp]
    dtt = np.ascontiguousarray(d16.T).reshape(FT, P, H)
    return gtt, utt, dtt


def kernel(x, router_w, gate_w, up_w, down_w):
    from concourse.bass_utils import run_bass_kernel_spmd

    x = np.asarray(x)
    router_w = np.asarray(router_w)
    gate_w = np.asarray(gate_w)
    up_w = np.asarray(up_w)
    down_w = np.asarray(down_w)

    xf = x.reshape(T, H)
    order, probs = _route(xf, router_w)

    # per-expert token lists + combine weights
    idxs, pes = [], []
    for e in range(E):
        sel = (order[:, 0] == e) | (order[:, 1] == e)
        idx = np.nonzero(sel)[0]
        pe = np.where(order[idx, 0] == e, probs[idx, 0], probs[idx, 1])
        idxs.append(idx)
        pes.append(pe.astype(np.float32))

    maxn = max(len(i) for i in idxs)
    npad = max(P, -(-maxn // P) * P)
    MT = npad // P

    if npad not in _nc_cache:
        _nc_cache[npad] = _build_nc(npad)
    nc = _nc_cache[npad]

    in_maps = []
    for e in range(E):
        idx, pe = idxs[e], pes[e]
        xg = np.zeros((npad, H), dtype=BF16)
        xg[: len(idx)] = xf[idx].astype(BF16)
        # [p, hc, t] = xg[t, hc*128+p]
        xtt = np.ascontiguousarray(xg.reshape(npad, HC, P).transpose(2, 1, 0))
        pp = np.zeros(npad, dtype=np.float32)
        pp[: len(idx)] = pe
        prt = np.ascontiguousarray(pp.reshape(MT, P).T)
        gtt, utt, dtt = _prep_expert_weights(gate_w, up_w, down_w, e)
        in_maps.append(
            {
                "xt": xtt,
                "gt": gtt,
                "ut": utt,
                "dw": dtt,
                "pr": prt,
                "wgsrc": np.zeros((1, 1), dtype=BF16),
            }
        )

    res = run_bass_kernel_spmd(
        nc, in_maps, core_ids=list(range(NCORES)), trace=TRACE
    )
    global LAST_RESULT
    LAST_RESULT = res

    out_flat = np.zeros((T, H), dtype=np.float32)
    for e in range(E):
        y = res.results[e]["out"].reshape(npad, H)
        out_flat[idxs[e]] += y[: len(idxs[e])]
    return out_flat.reshape(B, S, H)
